# revision 6
# baseline (speedup 1.0000x reference)
"""Conv2d 3x3 (B=32, Cin=128, H=W=56, Cout=256, pad=1, stride=1) + bias.

Strategy: data-parallel over batch across 8 NeuronCores (4 images/core).
Per core, implicit-GEMM conv with fp8-e4m3 DoubleRow matmuls: each tap is
ONE matmul contracting K=256 (two fp8 planes summed in-cell by the PE),
streaming 1 output element/cycle — 2x the fp16 MAC rate. LDWEIGHTS
(135ns for the 256-col fp8 load) is fully hidden under the 187ns matmul
stream, so per-matmul weight loads are free at this free-dim.

Precision: x and w are each sent as the average of two e4m3 roundings
(a = Q(v), b = Q(2v - a), so (a+b)/2 carries half the single-rounding
error). Plane 0 holds the a-copies, plane 1 the b-copies; DoubleRow
computes a_x*a_w + b_x*b_w ~= 2*x*w and the 0.5 is folded into the
bias stage. Measured rel-max error on the harness inputs: 1.62e-2
(gate 2e-2).

x is staged in SBUF as overlapping row bands [128, 2, 10, W] (one per
output row tile), so compute starts as soon as the first band lands.
Vertical padding = clipped matmul row windows at the image edges;
horizontal padding = clipped column sub-ranges with PSUM
first-touch-overwrite semantics.

Output path: PSUM -> SBUF fp16 with (0.5*acc + bias), alternating
between the scalar engine (activation) and the vector engine
(tensor_scalar); out DMAs ride the otherwise-idle gpsimd ring so their
~0.6us software issue cost doesn't serialize with the drains. fp16 DMA
out, upcast to fp32 on host.
"""

import numpy as np
import ml_dtypes

import concourse.bass as bass
import concourse.mybir as mybir
import concourse.tile as tile
from concourse import bacc
from concourse.bass_utils import run_bass_kernel_spmd

B, C_IN, H, W = 32, 128, 56, 56
C_OUT, KSZ = 256, 3
N_CORES = 8
B_LOC = B // N_CORES  # 4 images per core
RT = 8  # output rows per tile
NT = H // RT  # 7 row tiles
CBLKS = C_OUT // 128  # 2

E4 = ml_dtypes.float8_e4m3
MM_DT = mybir.dt.float8e4
DR = mybir.MatmulPerfMode.DoubleRow


def build_nc():
    nc = bacc.Bacc(None, target_bir_lowering=False)
    x = nc.dram_tensor("x", [B_LOC, C_IN, 2, H, W], MM_DT, kind="ExternalInput")
    wt = nc.dram_tensor(
        "wt", [C_IN, CBLKS, KSZ * KSZ, 2, 128], MM_DT, kind="ExternalInput"
    )
    bias = nc.dram_tensor("bias", [128, CBLKS], mybir.dt.float32, kind="ExternalInput")
    out = nc.dram_tensor("out", [B_LOC, C_OUT, H, W], mybir.dt.float16, kind="ExternalOutput")

    with tile.TileContext(nc) as tc:
        with (
            tc.tile_pool(name="xin", bufs=6) as xpool,
            tc.tile_pool(name="wpool", bufs=1) as wpool,
            tc.tile_pool(name="psum", bufs=7, space="PSUM") as psum_pool,
            tc.tile_pool(name="outp", bufs=6) as opool,
        ):
            # weights + bias on the scalar DMA ring (sync ring carries x bands)
            w_sb = wpool.tile([C_IN, CBLKS, KSZ * KSZ, 2, 128], MM_DT)
            for cb in range(CBLKS):
                nc.scalar.dma_start(w_sb[:, cb], wt[:, cb])
            bias_sb = wpool.tile([128, CBLKS], mybir.dt.float32)
            nc.scalar.dma_start(bias_sb[:], bias[:, :])

            # HAM pre-warm: dummy matmuls on a memset scratch tile, so the PE
            # ramps while the first x bands are still in flight.
            warm = wpool.tile([C_IN, 2, 256], MM_DT)
            warm_ps = psum_pool.tile([128, 256], mybir.dt.float32, name="warm_ps", bufs=1)
            nc.gpsimd.memset(warm[:].bitcast(mybir.dt.uint8), 0)
            for _ in range(10):
                nc.tensor.matmul(
                    warm_ps[:], warm[:, :, :128], warm[:, :, :256],
                    start=True, stop=True, perf_mode=DR, skip_group_check=True,
                )

            def band(b, t):
                """Stage x rows 8t-1 .. 8t+8 of image b as [128, 2, 10, W].
                Edge bands leave their pad row uninitialized; the matmul
                windows clip those rows instead of reading zeros."""
                xt = xpool.tile([C_IN, 2, RT + 2, W], MM_DT)
                r0 = max(0, t * RT - 1)
                r1 = min(H, t * RT + RT + 1)
                l0 = 1 if t == 0 else 0
                nc.sync.dma_start(
                    xt[:, :, l0 : l0 + (r1 - r0), :], x[b, :, :, r0:r1, :]
                )
                return xt

            ocnt = 0
            for b in range(B_LOC):
                for t in range(NT):
                    xt = band(b, t)
                    for cb in range(CBLKS):
                        ps = psum_pool.tile([128, RT, W], mybir.dt.float32)
                        for ky in range(KSZ):
                            # clip rows that would read the uninitialized
                            # pad row of the first/last band
                            r_off = 1 if (t == 0 and ky == 0) else 0
                            nrow = RT - r_off - (
                                1 if (t == NT - 1 and ky == 2) else 0
                            )
                            for kx in range(KSZ):
                                # clip columns at image edges
                                oc0 = 1 if kx == 0 else 0
                                ncol = W - (1 if kx != 1 else 0)
                                ic0 = 0 if kx == 0 else kx - 1
                                nc.tensor.matmul(
                                    ps[:, r_off : r_off + nrow, oc0 : oc0 + ncol],
                                    w_sb[:, cb, ky * KSZ + kx],
                                    xt[
                                        :,
                                        :,
                                        ky + r_off : ky + r_off + nrow,
                                        ic0 : ic0 + ncol,
                                    ],
                                    start=(ky == 0 and kx == 0),
                                    stop=(ky == 2 and kx == 2),
                                    perf_mode=DR,
                                    skip_group_check=True,
                                )
                        ot = opool.tile([128, RT, W], mybir.dt.float16)
                        # out = 0.5*acc + bias; alternate scalar/vector engines
                        if ocnt % 2 == 0:
                            nc.scalar.activation(
                                ot[:],
                                ps[:],
                                mybir.ActivationFunctionType.Identity,
                                bias=bias_sb[:, cb : cb + 1],
                                scale=0.5,
                            )
                        else:
                            nc.vector.tensor_scalar(
                                ot[:],
                                ps[:],
                                0.5,
                                bias_sb[:, cb : cb + 1],
                                mybir.AluOpType.mult,
                                mybir.AluOpType.add,
                            )
                        ocnt += 1
                        nc.gpsimd.dma_start(
                            out[b, cb * 128 : (cb + 1) * 128, t * RT : (t + 1) * RT, :],
                            ot[:],
                        )
    nc.finalize()
    return nc


def _two_round(v):
    """Return the two e4m3 roundings a=Q(v), b=Q(2v-a) with (a+b)/2 ~ v."""
    a = v.astype(E4)
    b = (2.0 * v - a.astype(np.float32)).astype(E4)
    return a, b


def prep_inputs(x, weight, bias):
    # weight (256,128,3,3) -> [ci, cb, tap, co_l] fp32
    wt = (
        weight.reshape(CBLKS, 128, C_IN, KSZ, KSZ)
        .transpose(2, 0, 3, 4, 1)
        .reshape(C_IN, CBLKS, KSZ * KSZ, 128)
        .astype(np.float32)
    )
    aw, bw = _two_round(wt)
    wt2 = np.ascontiguousarray(np.stack([aw, bw], axis=3))  # [ci,cb,tap,2,co]
    bias_r = np.ascontiguousarray(bias.reshape(CBLKS, 128).T, dtype=np.float32)
    ax, bx = _two_round(x.astype(np.float32))
    x2 = np.stack([ax, bx], axis=2)  # [B, Cin, 2, H, W]
    in_maps = []
    for c in range(N_CORES):
        in_maps.append(
            {
                "x": np.ascontiguousarray(x2[c * B_LOC : (c + 1) * B_LOC]),
                "wt": wt2,
                "bias": bias_r,
            }
        )
    return in_maps


_NC_CACHE = []


def run(x, weight, bias, trace=False, nc=None, tmpdir=None):
    if nc is None:
        if not _NC_CACHE:
            _NC_CACHE.append(build_nc())
        nc = _NC_CACHE[0]
    in_maps = prep_inputs(np.asarray(x), np.asarray(weight), np.asarray(bias))
    res = run_bass_kernel_spmd(
        nc, in_maps, core_ids=list(range(N_CORES)), trace=trace, tmpdir=tmpdir
    )
    out = np.concatenate([r["out"] for r in res.results], axis=0).astype(np.float32)
    return out, res


def kernel(x, weight, bias):
    out, _ = run(x, weight, bias, trace=False)
    return out


if __name__ == "__main__":
    rng = np.random.default_rng(0)
    x = rng.standard_normal((B, C_IN, H, W), dtype=np.float32)
    w = (rng.standard_normal((C_OUT, C_IN, KSZ, KSZ), dtype=np.float32) * 0.05).astype(
        np.float32
    )
    b = rng.standard_normal((C_OUT,), dtype=np.float32)
    out = kernel(x, w, b)
    print(out.shape, out.dtype)


# revision 7
# speedup vs baseline: 1.0473x; 1.0473x over previous
"""Conv2d 3x3 (B=32, Cin=128, H=W=56, Cout=256, pad=1, stride=1) + bias.

Strategy: data-parallel over batch across 8 NeuronCores (4 images/core).
Per core, implicit-GEMM conv with fp8-e4m3 DoubleRow matmuls: each tap is
ONE matmul contracting K=256 (two fp8 planes summed in-cell by the PE),
streaming 1 output element/cycle — 2x the fp16 MAC rate. LDWEIGHTS
(135ns for the 256-col fp8 load) is fully hidden under the 187ns matmul
stream, so per-matmul weight loads are free at this free-dim.

Precision: x and w are each sent as the average of two e4m3 roundings
(a = Q(v), b = Q(2v - a), so (a+b)/2 carries half the single-rounding
error). Plane 0 holds the a-copies, plane 1 the b-copies; DoubleRow
computes a_x*a_w + b_x*b_w ~= 2*x*w and the 0.5 is folded into the
bias stage. Measured rel-max error on the harness inputs: 1.62e-2
(gate 2e-2).

x is staged in SBUF as overlapping row bands [128, 2, 10, W] (one per
output row tile), so compute starts as soon as the first band lands.
Vertical padding = clipped matmul row windows at the image edges;
horizontal padding = clipped column sub-ranges with PSUM
first-touch-overwrite semantics.

Output path: PSUM -> SBUF fp16 with (0.5*acc + bias), alternating
between the scalar engine (activation) and the vector engine
(tensor_scalar); out DMAs alternate between the scalar and sync rings so their ~0.6us
software issue cost is split and doesn't serialize with the drains at
the kernel tail. fp16 DMA
out, upcast to fp32 on host.
"""

import numpy as np
import ml_dtypes

import concourse.bass as bass
import concourse.mybir as mybir
import concourse.tile as tile
from concourse import bacc
from concourse.bass_utils import run_bass_kernel_spmd

B, C_IN, H, W = 32, 128, 56, 56
C_OUT, KSZ = 256, 3
N_CORES = 8
B_LOC = B // N_CORES  # 4 images per core
RT = 8  # output rows per tile
NT = H // RT  # 7 row tiles
CBLKS = C_OUT // 128  # 2

E4 = ml_dtypes.float8_e4m3
MM_DT = mybir.dt.float8e4
DR = mybir.MatmulPerfMode.DoubleRow


def build_nc():
    nc = bacc.Bacc(None, target_bir_lowering=False)
    x = nc.dram_tensor("x", [B_LOC, C_IN, 2, H, W], MM_DT, kind="ExternalInput")
    wt = nc.dram_tensor(
        "wt", [C_IN, CBLKS, KSZ * KSZ, 2, 128], MM_DT, kind="ExternalInput"
    )
    bias = nc.dram_tensor("bias", [128, CBLKS], mybir.dt.float32, kind="ExternalInput")
    out = nc.dram_tensor("out", [B_LOC, C_OUT, H, W], mybir.dt.float16, kind="ExternalOutput")

    with tile.TileContext(nc) as tc:
        with (
            tc.tile_pool(name="xin", bufs=6) as xpool,
            tc.tile_pool(name="wpool", bufs=1) as wpool,
            tc.tile_pool(name="psum", bufs=7, space="PSUM") as psum_pool,
            tc.tile_pool(name="outp", bufs=6) as opool,
        ):
            # weights + bias on the scalar DMA ring (sync ring carries x bands)
            w_sb = wpool.tile([C_IN, CBLKS, KSZ * KSZ, 2, 128], MM_DT)
            for cb in range(CBLKS):
                nc.scalar.dma_start(w_sb[:, cb], wt[:, cb])
            bias_sb = wpool.tile([128, CBLKS], mybir.dt.float32)
            nc.scalar.dma_start(bias_sb[:], bias[:, :])

            # HAM pre-warm: dummy matmuls on a memset scratch tile, so the PE
            # ramps while the first x bands are still in flight.
            warm = wpool.tile([C_IN, 2, 256], MM_DT)
            warm_ps = psum_pool.tile([128, 256], mybir.dt.float32, name="warm_ps", bufs=1)
            nc.gpsimd.memset(warm[:].bitcast(mybir.dt.uint8), 0)
            for _ in range(16):
                nc.tensor.matmul(
                    warm_ps[:], warm[:, :, :128], warm[:, :, :256],
                    start=True, stop=True, perf_mode=DR, skip_group_check=True,
                )

            def band(b, t):
                """Stage x rows 8t-1 .. 8t+8 of image b as [128, 2, 10, W].
                Edge bands leave their pad row uninitialized; the matmul
                windows clip those rows instead of reading zeros."""
                xt = xpool.tile([C_IN, 2, RT + 2, W], MM_DT)
                r0 = max(0, t * RT - 1)
                r1 = min(H, t * RT + RT + 1)
                l0 = 1 if t == 0 else 0
                nc.sync.dma_start(
                    xt[:, :, l0 : l0 + (r1 - r0), :], x[b, :, :, r0:r1, :]
                )
                return xt

            ocnt = 0
            for b in range(B_LOC):
                for t in range(NT):
                    xt = band(b, t)
                    for cb in range(CBLKS):
                        ps = psum_pool.tile([128, RT, W], mybir.dt.float32)
                        for ky in range(KSZ):
                            # clip rows that would read the uninitialized
                            # pad row of the first/last band
                            r_off = 1 if (t == 0 and ky == 0) else 0
                            nrow = RT - r_off - (
                                1 if (t == NT - 1 and ky == 2) else 0
                            )
                            for kx in range(KSZ):
                                # clip columns at image edges
                                oc0 = 1 if kx == 0 else 0
                                ncol = W - (1 if kx != 1 else 0)
                                ic0 = 0 if kx == 0 else kx - 1
                                nc.tensor.matmul(
                                    ps[:, r_off : r_off + nrow, oc0 : oc0 + ncol],
                                    w_sb[:, cb, ky * KSZ + kx],
                                    xt[
                                        :,
                                        :,
                                        ky + r_off : ky + r_off + nrow,
                                        ic0 : ic0 + ncol,
                                    ],
                                    start=(ky == 0 and kx == 0),
                                    stop=(ky == 2 and kx == 2),
                                    perf_mode=DR,
                                    skip_group_check=True,
                                )
                        ot = opool.tile([128, RT, W], mybir.dt.float16)
                        # out = 0.5*acc + bias; alternate scalar/vector engines
                        if ocnt % 2 == 0:
                            nc.scalar.activation(
                                ot[:],
                                ps[:],
                                mybir.ActivationFunctionType.Identity,
                                bias=bias_sb[:, cb : cb + 1],
                                scale=0.5,
                            )
                        else:
                            nc.vector.tensor_scalar(
                                ot[:],
                                ps[:],
                                0.5,
                                bias_sb[:, cb : cb + 1],
                                mybir.AluOpType.mult,
                                mybir.AluOpType.add,
                            )
                        ocnt += 1
                        dma_eng = nc.scalar if ocnt % 2 == 0 else nc.sync
                        dma_eng.dma_start(
                            out[b, cb * 128 : (cb + 1) * 128, t * RT : (t + 1) * RT, :],
                            ot[:],
                        )
    nc.finalize()
    return nc


def _two_round(v):
    """Return the two e4m3 roundings a=Q(v), b=Q(2v-a) with (a+b)/2 ~ v."""
    a = v.astype(E4)
    b = (2.0 * v - a.astype(np.float32)).astype(E4)
    return a, b


def prep_inputs(x, weight, bias):
    # weight (256,128,3,3) -> [ci, cb, tap, co_l] fp32
    wt = (
        weight.reshape(CBLKS, 128, C_IN, KSZ, KSZ)
        .transpose(2, 0, 3, 4, 1)
        .reshape(C_IN, CBLKS, KSZ * KSZ, 128)
        .astype(np.float32)
    )
    aw, bw = _two_round(wt)
    wt2 = np.ascontiguousarray(np.stack([aw, bw], axis=3))  # [ci,cb,tap,2,co]
    bias_r = np.ascontiguousarray(bias.reshape(CBLKS, 128).T, dtype=np.float32)
    ax, bx = _two_round(x.astype(np.float32))
    x2 = np.stack([ax, bx], axis=2)  # [B, Cin, 2, H, W]
    in_maps = []
    for c in range(N_CORES):
        in_maps.append(
            {
                "x": np.ascontiguousarray(x2[c * B_LOC : (c + 1) * B_LOC]),
                "wt": wt2,
                "bias": bias_r,
            }
        )
    return in_maps


_NC_CACHE = []


def run(x, weight, bias, trace=False, nc=None, tmpdir=None):
    if nc is None:
        if not _NC_CACHE:
            _NC_CACHE.append(build_nc())
        nc = _NC_CACHE[0]
    in_maps = prep_inputs(np.asarray(x), np.asarray(weight), np.asarray(bias))
    res = run_bass_kernel_spmd(
        nc, in_maps, core_ids=list(range(N_CORES)), trace=trace, tmpdir=tmpdir
    )
    out = np.concatenate([r["out"] for r in res.results], axis=0).astype(np.float32)
    return out, res


def kernel(x, weight, bias):
    out, _ = run(x, weight, bias, trace=False)
    return out


if __name__ == "__main__":
    rng = np.random.default_rng(0)
    x = rng.standard_normal((B, C_IN, H, W), dtype=np.float32)
    w = (rng.standard_normal((C_OUT, C_IN, KSZ, KSZ), dtype=np.float32) * 0.05).astype(
        np.float32
    )
    b = rng.standard_normal((C_OUT,), dtype=np.float32)
    out = kernel(x, w, b)
    print(out.shape, out.dtype)


# revision 8
# speedup vs baseline: 1.0545x; 1.0069x over previous
"""Conv2d 3x3 (B=32, Cin=128, H=W=56, Cout=256, pad=1, stride=1) + bias.

Strategy: data-parallel over batch across 8 NeuronCores (4 images/core).
Per core, implicit-GEMM conv with fp8-e4m3 DoubleRow matmuls: each tap is
ONE matmul contracting K=256 (two fp8 planes summed in-cell by the PE),
streaming 1 output element/cycle — 2x the fp16 MAC rate. LDWEIGHTS
(135ns for the 256-col fp8 load) is fully hidden under the 187ns matmul
stream, so per-matmul weight loads are free at this free-dim.

Precision: x and w are each sent as the average of two e4m3 roundings
(a = Q(v), b = Q(2v - a), so (a+b)/2 carries half the single-rounding
error). Plane 0 holds the a-copies, plane 1 the b-copies; DoubleRow
computes a_x*a_w + b_x*b_w ~= 2*x*w and the 0.5 is folded into the
bias stage. Measured rel-max error on the harness inputs: 1.62e-2
(gate 2e-2).

x is staged in SBUF as overlapping row bands [128, 2, 10, W] (one per
output row tile), so compute starts as soon as the first band lands.
Vertical padding = clipped matmul row windows at the image edges;
horizontal padding = clipped column sub-ranges with PSUM
first-touch-overwrite semantics.

Output path: PSUM -> SBUF fp16 with (0.5*acc + bias), alternating
between the scalar engine (activation) and the vector engine
(tensor_scalar); out DMAs alternate between the scalar and sync rings so their ~0.6us
software issue cost is split and doesn't serialize with the drains at
the kernel tail. fp16 DMA
out, upcast to fp32 on host.
"""

import numpy as np
import ml_dtypes

import concourse.bass as bass
import concourse.mybir as mybir
import concourse.tile as tile
from concourse import bacc
from concourse.bass_utils import run_bass_kernel_spmd

B, C_IN, H, W = 32, 128, 56, 56
C_OUT, KSZ = 256, 3
N_CORES = 8
B_LOC = B // N_CORES  # 4 images per core
RT = 8  # output rows per tile
NT = H // RT  # 7 row tiles
CBLKS = C_OUT // 128  # 2

E4 = ml_dtypes.float8_e4m3
MM_DT = mybir.dt.float8e4
DR = mybir.MatmulPerfMode.DoubleRow


def build_nc():
    nc = bacc.Bacc(None, target_bir_lowering=False)
    x = nc.dram_tensor("x", [B_LOC, C_IN, 2, H, W], MM_DT, kind="ExternalInput")
    wt = nc.dram_tensor(
        "wt", [C_IN, CBLKS, KSZ * KSZ, 2, 128], MM_DT, kind="ExternalInput"
    )
    bias = nc.dram_tensor("bias", [128, CBLKS], mybir.dt.float32, kind="ExternalInput")
    out = nc.dram_tensor("out", [B_LOC, C_OUT, H, W], mybir.dt.float16, kind="ExternalOutput")

    with tile.TileContext(nc) as tc:
        with (
            tc.tile_pool(name="xin", bufs=6) as xpool,
            tc.tile_pool(name="wpool", bufs=1) as wpool,
            tc.tile_pool(name="psum", bufs=7, space="PSUM") as psum_pool,
            tc.tile_pool(name="outp", bufs=6) as opool,
        ):
            # weights + bias on the scalar DMA ring (sync ring carries x bands)
            w_sb = wpool.tile([C_IN, CBLKS, KSZ * KSZ, 2, 128], MM_DT)
            for cb in range(CBLKS):
                nc.scalar.dma_start(w_sb[:, cb], wt[:, cb])
            bias_sb = wpool.tile([128, CBLKS], mybir.dt.float32)
            nc.scalar.dma_start(bias_sb[:], bias[:, :])

            # HAM pre-warm: dummy matmuls on a memset scratch tile, so the PE
            # ramps while the first x bands are still in flight.
            warm = wpool.tile([C_IN, 2, 256], MM_DT)
            warm_ps = psum_pool.tile([128, 256], mybir.dt.float32, name="warm_ps", bufs=1)
            nc.gpsimd.memset(warm[:].bitcast(mybir.dt.uint8), 0)
            for _ in range(16):
                nc.tensor.matmul(
                    warm_ps[:], warm[:, :, :128], warm[:, :, :256],
                    start=True, stop=True, perf_mode=DR, skip_group_check=True,
                )

            def band(b, t):
                """Stage x rows 8t-1 .. 8t+8 of image b as [128, 2, 10, W].
                Edge bands leave their pad row uninitialized; the matmul
                windows clip those rows instead of reading zeros."""
                xt = xpool.tile([C_IN, 2, RT + 2, W], MM_DT)
                r0 = max(0, t * RT - 1)
                r1 = min(H, t * RT + RT + 1)
                l0 = 1 if t == 0 else 0
                nc.sync.dma_start(
                    xt[:, :, l0 : l0 + (r1 - r0), :], x[b, :, :, r0:r1, :]
                )
                return xt

            ocnt = 0
            for b in range(B_LOC):
                for t in range(NT):
                    xt = band(b, t)
                    for cb in range(CBLKS):
                        ps = psum_pool.tile([128, RT, W], mybir.dt.float32)
                        for ky in range(KSZ):
                            # clip rows that would read the uninitialized
                            # pad row of the first/last band
                            r_off = 1 if (t == 0 and ky == 0) else 0
                            nrow = RT - r_off - (
                                1 if (t == NT - 1 and ky == 2) else 0
                            )
                            for kx in range(KSZ):
                                # clip columns at image edges
                                oc0 = 1 if kx == 0 else 0
                                ncol = W - (1 if kx != 1 else 0)
                                ic0 = 0 if kx == 0 else kx - 1
                                nc.tensor.matmul(
                                    ps[:, r_off : r_off + nrow, oc0 : oc0 + ncol],
                                    w_sb[:, cb, ky * KSZ + kx],
                                    xt[
                                        :,
                                        :,
                                        ky + r_off : ky + r_off + nrow,
                                        ic0 : ic0 + ncol,
                                    ],
                                    start=(ky == 0 and kx == 0),
                                    stop=(ky == 2 and kx == 2),
                                    perf_mode=DR,
                                    skip_group_check=True,
                                )
                        ot = opool.tile([128, RT, W], mybir.dt.float16)
                        last = b == B_LOC - 1 and t == NT - 1 and cb == CBLKS - 1
                        if last:
                            # split the final drain into two parallel halves
                            # (scalar || vector, DMAs on two rings) to shorten
                            # the kernel tail
                            hr = RT // 2
                            nc.scalar.activation(
                                ot[:, :hr],
                                ps[:, :hr],
                                mybir.ActivationFunctionType.Identity,
                                bias=bias_sb[:, cb : cb + 1],
                                scale=0.5,
                            )
                            nc.vector.tensor_scalar(
                                ot[:, hr:],
                                ps[:, hr:],
                                0.5,
                                bias_sb[:, cb : cb + 1],
                                mybir.AluOpType.mult,
                                mybir.AluOpType.add,
                            )
                            orow = t * RT
                            nc.sync.dma_start(
                                out[b, cb * 128 : (cb + 1) * 128, orow : orow + hr, :],
                                ot[:, :hr],
                            )
                            nc.scalar.dma_start(
                                out[b, cb * 128 : (cb + 1) * 128, orow + hr : orow + RT, :],
                                ot[:, hr:],
                            )
                            ocnt += 1
                            continue
                        # out = 0.5*acc + bias; alternate scalar/vector engines
                        if ocnt % 2 == 0:
                            nc.scalar.activation(
                                ot[:],
                                ps[:],
                                mybir.ActivationFunctionType.Identity,
                                bias=bias_sb[:, cb : cb + 1],
                                scale=0.5,
                            )
                        else:
                            nc.vector.tensor_scalar(
                                ot[:],
                                ps[:],
                                0.5,
                                bias_sb[:, cb : cb + 1],
                                mybir.AluOpType.mult,
                                mybir.AluOpType.add,
                            )
                        ocnt += 1
                        dma_eng = nc.scalar if ocnt % 2 == 0 else nc.sync
                        dma_eng.dma_start(
                            out[b, cb * 128 : (cb + 1) * 128, t * RT : (t + 1) * RT, :],
                            ot[:],
                        )
    nc.finalize()
    return nc


def _two_round(v):
    """Return the two e4m3 roundings a=Q(v), b=Q(2v-a) with (a+b)/2 ~ v."""
    a = v.astype(E4)
    b = (2.0 * v - a.astype(np.float32)).astype(E4)
    return a, b


def prep_inputs(x, weight, bias):
    # weight (256,128,3,3) -> [ci, cb, tap, co_l] fp32
    wt = (
        weight.reshape(CBLKS, 128, C_IN, KSZ, KSZ)
        .transpose(2, 0, 3, 4, 1)
        .reshape(C_IN, CBLKS, KSZ * KSZ, 128)
        .astype(np.float32)
    )
    aw, bw = _two_round(wt)
    wt2 = np.ascontiguousarray(np.stack([aw, bw], axis=3))  # [ci,cb,tap,2,co]
    bias_r = np.ascontiguousarray(bias.reshape(CBLKS, 128).T, dtype=np.float32)
    ax, bx = _two_round(x.astype(np.float32))
    x2 = np.stack([ax, bx], axis=2)  # [B, Cin, 2, H, W]
    in_maps = []
    for c in range(N_CORES):
        in_maps.append(
            {
                "x": np.ascontiguousarray(x2[c * B_LOC : (c + 1) * B_LOC]),
                "wt": wt2,
                "bias": bias_r,
            }
        )
    return in_maps


_NC_CACHE = []


def run(x, weight, bias, trace=False, nc=None, tmpdir=None):
    if nc is None:
        if not _NC_CACHE:
            _NC_CACHE.append(build_nc())
        nc = _NC_CACHE[0]
    in_maps = prep_inputs(np.asarray(x), np.asarray(weight), np.asarray(bias))
    res = run_bass_kernel_spmd(
        nc, in_maps, core_ids=list(range(N_CORES)), trace=trace, tmpdir=tmpdir
    )
    out = np.concatenate([r["out"] for r in res.results], axis=0).astype(np.float32)
    return out, res


def kernel(x, weight, bias):
    out, _ = run(x, weight, bias, trace=False)
    return out


if __name__ == "__main__":
    rng = np.random.default_rng(0)
    x = rng.standard_normal((B, C_IN, H, W), dtype=np.float32)
    w = (rng.standard_normal((C_OUT, C_IN, KSZ, KSZ), dtype=np.float32) * 0.05).astype(
        np.float32
    )
    b = rng.standard_normal((C_OUT,), dtype=np.float32)
    out = kernel(x, w, b)
    print(out.shape, out.dtype)
